# revision 12
# baseline (speedup 1.0000x reference)
"""Trainium2 Bass kernel for fused MHA with q/k std-normalization. v3.

Reference computation (per batch b, head h):
    q,k,v = x[b].T @ Wq/Wk/Wv          [T, 64] each
    q = (q - mean_e) / (std_e(ddof=1) + 1e-5)   (same for k)
    attn = softmax(q @ k.T / 8)
    out[b, h*64:(h+1)*64, :] = (attn @ v).T

Sharding: 8 cores = 4 batches x 2 half-head-groups. Core c handles batch
c//2, heads (c%2)*8 .. +8. Fully head-independent, no collectives.

Design notes (v1 = 568us was PE-p-state-bound: TRN2's PE only reaches
2.4GHz after ~3us of continuous execution, and any cross-engine stall
drops it back to 1.2GHz; the whole game is an uninterrupted PE stream):
  * All matmuls bf16 (1 cyc/row, 0.37ns/row measured). fp8/DoubleRow was
    measured at ~1.8e-2 rel err through softmax*V - over tolerance.
  * Host-side: q/k weight columns mean-centered (mean subtraction free,
    std translation-invariant); the +1e-5 on sigma dropped (rel 2e-5).
  * Phase 1 per (t-tile, head-pair): psum released immediately by two
    ACT copies (v -> bf16 tiles, raw q~k~ -> bf16 staging); variance =
    4x DVE square+accum on the staging; normalize = DVE tensor_scalar
    with per-partition 1/sigma; PE transposes are SOFTWARE-PIPELINED
    one t-tile behind so their stats chain is long resolved when the
    PE reaches them.
  * Phase 2 per (head, strip): scores K=64 bf16; exp split by columns:
    ACT true Exp -> bf16, DVE writes the bf16 BITPATTERN via one
    mult+add into int16 (Schraudolph, RNE+saturation verified); PV
    accumulates [65,512] with a ones-column in the v tiles producing
    the softmax denominator; scores run LOOKAHEAD=2 chunks ahead of PV
    so exp latency never stalls the PE. The reciprocal broadcast psum
    reuses the PV pool's tag (8-bank budget).
"""

import sys

if "/opt/trn_rl_repo" not in sys.path:
    sys.path.insert(0, "/opt/trn_rl_repo")

import numpy as np
import ml_dtypes

B, D, T, H = 4, 1024, 2048, 16
NHL = 8            # heads per core
DH = 64            # head dim
NT = T // 128      # 16 t-tiles
ND = D // 128      # 8 d-chunks
NST = T // 512     # 4 t-strips

EXPC = 1.6                                # exp shift: w = exp(s/8 - EXPC)
LOG2E = float(np.log2(np.e))
SCH_A = 0.125 * LOG2E * 128.0             # 23.0831
SCH_B = (127.0 - EXPC * LOG2E) * 128.0 - 7.4   # RNE balance tune
ACT_C = 576                               # exp cols on ACT (rest DVE)

_prog = None


def _build(loop_n=None, part=None):
    import contextlib
    import concourse.bass as bass
    import concourse.bacc as bacc
    import concourse.tile as tile
    from concourse import mybir
    from concourse.masks import make_identity

    f32 = mybir.dt.float32
    f32r = mybir.dt.float32r
    bf16 = mybir.dt.bfloat16
    i16 = mybir.dt.int16
    AF = mybir.ActivationFunctionType
    ALU = mybir.AluOpType

    nc = bacc.Bacc()
    x_ext = nc.dram_tensor("x_local", [NT, ND, 128, 128], bf16, kind="ExternalInput")
    w_ext = nc.dram_tensor("w_local", [ND, 128, NHL * 192], bf16, kind="ExternalInput")
    out_ext = nc.dram_tensor("out_local", [NHL * DH, T], f32, kind="ExternalOutput")

    with tile.TileContext(nc) as tc:
      with (tc.For_i(0, loop_n, 1) if loop_n else contextlib.nullcontext()):
          with tc.tile_pool(name="persist", bufs=1) as persist, \
               tc.tile_pool(name="qkTp", bufs=1) as qkTp, \
               tc.tile_pool(name="vp", bufs=1) as vp:
              identb = persist.tile([128, 128], bf16, tag="identb")
              make_identity(nc, identb)
              ones65 = persist.tile([1, 65], f32r, tag="ones65")
              nc.vector.memset(ones65.bitcast(f32), 1.0)
              biasx = persist.tile([128, 1], f32, tag="biasx")
              nc.vector.memset(biasx, -EXPC)

              # qT[h]/kT[h]: [128, T] bf16; data on partitions 0:64,
              # partitions 64:128 zeroed so scores contract K=128 (K=64
              # matmuls measured 2.6x slower per row)
              qT = [qkTp.tile([128, T], bf16, tag=f"qT{h}", name=f"qT{h}")
                    for h in range(NHL)]
              kT = [qkTp.tile([128, T], bf16, tag=f"kT{h}", name=f"kT{h}")
                    for h in range(NHL)]
              for h in range(NHL):
                  nc.vector.memset(qT[h][64:128, :], 0.0)
                  nc.vector.memset(kT[h][64:128, :], 0.0)
              # vtp[hp][sc]: [128, 130] bf16 = [1 | v(2hp) | v(2hp+1) | 1].
              # PV lhsT: head even = [:, 0:65] (denominator psum row 0,
              # v rows 1:65), head odd = [:, 65:130] (v rows 0:64,
              # denominator row 64). One [128,128] evacuation per pair.
              vtp = [[vp.tile([128, 130], bf16, tag=f"v{hp}_{sc}",
                              name=f"v{hp}_{sc}")
                      for sc in range(NT)] for hp in range(NHL // 2)]
              for hp in range(NHL // 2):
                  for sc in range(NT):
                      nc.vector.memset(vtp[hp][sc][:, 0:1], 1.0)
                      nc.vector.memset(vtp[hp][sc][:, 129:130], 1.0)

              if part == "p2":
                  for h in range(NHL):
                      nc.vector.memset(qT[h], 0.01)
                      nc.vector.memset(kT[h], 0.01)
                  for hp in range(NHL // 2):
                      for sc in range(NT):
                          nc.vector.memset(vtp[hp][sc][:, 1:129], 0.01)

              # ---------------- Phase 1: projection + norm + transpose
              with tc.tile_pool(name="wsb", bufs=1) as wpool, \
                   tc.tile_pool(name="xin", bufs=2) as xpool, \
                   tc.tile_pool(name="stage", bufs=18) as stpool, \
                   tc.tile_pool(name="stats", bufs=8) as statp, \
                   tc.tile_pool(name="psumG", bufs=2, space="PSUM") as pgp, \
                   tc.tile_pool(name="trps", bufs=3, space="PSUM") as trp:
                  wsb = [wpool.tile([128, NHL * 192], bf16, tag=f"w{dc}",
                                    name=f"w{dc}")
                         for dc in range(ND)]
                  for dc in range(ND if part != "p2" else 0):
                      nc.sync.dma_start(out=wsb[dc], in_=w_ext[dc])

                  # pending transpose work, deferred one t-tile so the
                  # stats->norm chain is resolved before the PE needs it
                  pending = []

                  def flush(items):
                      for (stg_t, tih) in items:
                          tr = trp.tile([64, 256], bf16, tag="tr",
                                        name="tr")
                          nc.tensor.transpose(tr[:, 0:128], stg_t[:, 0:64],
                                              identb)
                          nc.tensor.transpose(tr[:, 128:256],
                                              stg_t[:, 64:128], identb)
                          ti_t = tih >> 3
                          hh = tih & 7
                          nc.scalar.copy(
                              qT[hh][0:64, ti_t * 128: (ti_t + 1) * 128],
                              tr[:, 0:128])
                          nc.vector.tensor_copy(
                              kT[hh][0:64, ti_t * 128: (ti_t + 1) * 128],
                              tr[:, 128:256])

                  for ti in range(NT if part != "p2" else 0):
                      xts = []
                      for dc in range(ND):
                          xt = xpool.tile([128, 128], bf16, tag=f"x{dc}",
                                          name=f"x{dc}")
                          nc.sync.dma_start(out=xt, in_=x_ext[ti, dc])
                          xts.append(xt)
                      nextp = []
                      for half in range(2):
                          ps = pgp.tile([128, 1024], f32, tag="pg")
                          for dc in range(ND):
                              for g2 in range(2):
                                  hp = half * 2 + g2
                                  nc.tensor.matmul(
                                      ps[:, g2 * 512: g2 * 512 + 384],
                                      lhsT=xts[dc],
                                      rhs=wsb[dc][:, hp * 384: (hp + 1) * 384],
                                      start=(dc == 0),
                                      stop=(dc == ND - 1),
                                  )
                          for g2 in range(2):
                              hp = half * 2 + g2
                              base = g2 * 512
                              # release psum fast: v + raw q~k~ -> bf16
                              nc.scalar.copy(
                                  vtp[hp][ti][:, 1:129],
                                  ps[:, base + 256: base + 384])
                              stg2 = stpool.tile([128, 256], bf16,
                                                 tag="stg2")
                              nc.scalar.copy(stg2, ps[:, base: base + 256])
                              # sumsq via DVE square+accum (per 64-col grp)
                              ss = statp.tile([128, 4], f32, tag="ss")
                              sqd = statp.tile([128, 256], bf16, tag="sqd")
                              for g in range(4):
                                  nc.vector.scalar_tensor_tensor(
                                      out=sqd[:, g * 64: (g + 1) * 64],
                                      in0=stg2[:, g * 64: (g + 1) * 64],
                                      scalar=1.0,
                                      in1=stg2[:, g * 64: (g + 1) * 64],
                                      op0=ALU.mult, op1=ALU.mult,
                                      accum_out=ss[:, g: g + 1])
                              # sigma = sqrt(ss/63); inv = 1/sigma
                              sg = statp.tile([128, 4], f32, tag="sg")
                              nc.scalar.activation(sg, ss, AF.Sqrt,
                                                   scale=float(1.0 / 63.0))
                              inv = statp.tile([128, 4], f32, tag="inv")
                              nc.vector.reciprocal(inv, sg)
                              for m in range(2):
                                  h = hp * 2 + m
                                  stg = stpool.tile([128, 128], bf16,
                                                    tag="stg")
                                  nc.vector.tensor_scalar(
                                      out=stg[:, 0:64],
                                      in0=stg2[:, m * 128: m * 128 + 64],
                                      scalar1=inv[:, 2 * m: 2 * m + 1],
                                      scalar2=None, op0=ALU.mult)
                                  nc.vector.tensor_scalar(
                                      out=stg[:, 64:128],
                                      in0=stg2[:, m * 128 + 64:
                                               m * 128 + 128],
                                      scalar1=inv[:, 2 * m + 1: 2 * m + 2],
                                      scalar2=None, op0=ALU.mult)
                                  nextp.append((stg, (ti << 3) | h))
                          # flush half of the previous t-tile's transposes
                          if pending:
                              flush(pending[len(pending) // 2:]
                                    if half else pending[: len(pending) // 2])
                      pending = nextp
                  flush(pending)
                  pending = []

              if part == "p1":
                  dummy = persist.tile([64, 512], f32, tag="dummy")
                  nc.vector.memset(dummy, 0.0)
                  nc.sync.dma_start(out=out_ext[0:64, 0:512], in_=dummy)

              # ---------------- Phase 2: attention per head
              with tc.tile_pool(name="pt", bufs=4) as ptp, \
                   tc.tile_pool(name="osb", bufs=5) as osbp, \
                   tc.tile_pool(name="outsb", bufs=3) as outp, \
                   tc.tile_pool(name="dt", bufs=2) as dtp, \
                   tc.tile_pool(name="spsum", bufs=3, space="PSUM") as spp, \
                   tc.tile_pool(name="opsum", bufs=2, space="PSUM") as opp:
                  for h in range(NHL if part != "p1" else 0):
                      hp2, mh = h // 2, h % 2
                      dt = dtp.tile([4, 512], f32, tag="dt")
                      rt = dtp.tile([4, 512], f32, tag="rt")
                      rtf = dtp.tile([1, 4 * 512], f32r, tag="rtf")
                      osbs = []
                      for st in range(NST):
                          op_ps = opp.tile([65, 512], f32, tag="op")
                          NJ = 8
                          LOOKAHEAD = 2
                          pts = []

                          def emit_scores(j):
                              sp = spp.tile([128, 1024], f32, tag="sp",
                                            name=f"sp{j}")
                              for u in range(2):
                                  sc = 2 * j + u
                                  nc.tensor.matmul(
                                      sp[:, u * 512: (u + 1) * 512],
                                      lhsT=kT[h][:, sc * 128: (sc + 1) * 128],
                                      rhs=qT[h][:, st * 512: (st + 1) * 512],
                                      start=True, stop=True)
                              pt = ptp.tile([128, 1024], bf16, tag="pt",
                                            name=f"pt{j}")
                              if part != "noexp":
                                  c0 = ACT_C
                                  nc.scalar.activation(
                                      pt[:, 0:c0], sp[:, 0:c0], AF.Exp,
                                      scale=0.125, bias=biasx)
                                  nc.vector.tensor_scalar(
                                      out=pt.bitcast(i16)[:, c0:1024],
                                      in0=sp[:, c0:1024],
                                      scalar1=float(SCH_A),
                                      scalar2=float(SCH_B),
                                      op0=ALU.mult, op1=ALU.add)
                              else:
                                  nc.vector.memset(pt[:, 0:1], 1.0)
                              pts.append(pt)

                          def emit_pv(j):
                              if part == "nopv":
                                  if j == 0:
                                      nc.vector.memset(op_ps[:, 0:1], 1.0)
                                  return
                              for u in range(2):
                                  sc = 2 * j + u
                                  nc.tensor.matmul(
                                      op_ps,
                                      lhsT=vtp[hp2][sc][:, mh * 65:
                                                        mh * 65 + 65],
                                      rhs=pts[j][:, u * 512: (u + 1) * 512],
                                      start=(sc == 0), stop=(sc == NT - 1))

                          for j in range(LOOKAHEAD):
                              emit_scores(j)
                          for j in range(NJ):
                              if j + LOOKAHEAD < NJ:
                                  emit_scores(j + LOOKAHEAD)
                              emit_pv(j)
                          osb = osbp.tile([65, 512], f32, tag="osb")
                          nc.scalar.copy(osb, op_ps)
                          drow = 0 if mh == 0 else 64
                          nc.sync.dma_start(out=dt[st: st + 1, :],
                                            in_=osb[drow: drow + 1, :])
                          osbs.append(osb)
                      nc.vector.reciprocal(rt, dt)
                      # flatten [4,512] partitions into partition 0's free
                      # dim; gpsimd DMA casts f32 -> f32r
                      nc.gpsimd.dma_start(out=rtf[0:1, :], in_=rt)
                      for st in range(NST):
                          # reciprocal broadcast; shares the opp tag so the
                          # PSUM budget stays within 8 banks
                          rep = opp.tile([65, 512], f32, tag="op")
                          nc.tensor.matmul(
                              rep, lhsT=ones65,
                              rhs=rtf[0:1, st * 512: (st + 1) * 512],
                              start=True, stop=True)
                          outt = outp.tile([65, 512], f32, tag="outt")
                          nc.vector.tensor_mul(outt, osbs[st], rep)
                          vr0 = 1 if mh == 0 else 0
                          nc.sync.dma_start(
                              out=out_ext[h * 64: (h + 1) * 64,
                                          st * 512: (st + 1) * 512],
                              in_=outt[vr0: vr0 + 64, :])
    nc.finalize()
    return nc


def _get_prog():
    global _prog
    if _prog is None:
        _prog = _build()
    return _prog


def make_in_maps(x, qkv):
    x = np.ascontiguousarray(np.asarray(x, dtype=np.float32))
    qkv = np.asarray(qkv, dtype=np.float32).copy()
    # center q/k weight columns over e (mean subtraction becomes free; std
    # is translation invariant)
    qkv[0] -= qkv[0].mean(axis=-1, keepdims=True)
    qkv[1] -= qkv[1].mean(axis=-1, keepdims=True)
    in_maps = []
    for c in range(8):
        b = c // 2
        hs = (c % 2) * 8
        # x: [16 ti, 8 dc, 128 dp, 128 tf]
        xp = np.ascontiguousarray(
            x[b].reshape(ND, 128, NT, 128).transpose(2, 0, 1, 3)
        ).astype(ml_dtypes.bfloat16)
        # w: per head pair hp (4): [qc(2hp) 64 | kc(2hp) 64 | qc(2hp+1) 64 |
        #    kc(2hp+1) 64 | v(2hp) 64 | v(2hp+1) 64]  -> 384 cols
        wp = np.empty((D, NHL * 192), np.float32)
        for hp in range(NHL // 2):
            h0, h1 = hs + 2 * hp, hs + 2 * hp + 1
            cb = hp * 384
            wp[:, cb + 0: cb + 64] = qkv[0, h0]
            wp[:, cb + 64: cb + 128] = qkv[1, h0]
            wp[:, cb + 128: cb + 192] = qkv[0, h1]
            wp[:, cb + 192: cb + 256] = qkv[1, h1]
            wp[:, cb + 256: cb + 320] = qkv[2, h0]
            wp[:, cb + 320: cb + 384] = qkv[2, h1]
        wp = wp.reshape(ND, 128, NHL * 192).astype(ml_dtypes.bfloat16)
        in_maps.append({"x_local": xp, "w_local": wp})
    return in_maps


def gather(results):
    out = np.empty((B, D, T), np.float32)
    for c in range(8):
        out[c // 2, (c % 2) * 512: (c % 2) * 512 + 512, :] = \
            results[c]["out_local"]
    return out


def kernel(**inputs):
    from concourse.bass_utils import run_bass_kernel_spmd

    nc = _get_prog()
    in_maps = make_in_maps(inputs["x"], inputs["qkv"])
    res = run_bass_kernel_spmd(nc, in_maps, list(range(8)))
    return gather(res.results)
